# revision 21
# baseline (speedup 1.0000x reference)
"""Bahdanau attention fused kernel for Trainium2, 8-core data-parallel.

Reference computation (per batch b of 32, H=1024, S=2048):
    enc_score = encoder_out @ We + be                    [B, S, H]
    dec_score = dec @ Wd + bd                            [B, 1, H]
    score     = tanh(enc_score + dec_score)              [B, S, H]
    ls        = score @ Ws + bs                          [B, S, 1]
    w         = softmax(ls, axis=S)
    out       = sum_s w[b,s] * encoder_out[b,s,:]        [B, H]

Sharding: batch 32 -> 4 per core across 8 cores; weights replicated.
The tiny dec-score GEMM is folded into the host-side bias preparation:
bias[b] = be + bd + dec[b] @ Wd. bs is dropped (softmax shift-invariant).

Numerics: the main GEMM and the ls projection run in fp8-e4m3 with
MatmulPerfMode.DoubleRow (two 128-deep k-tiles per matmul). We and Ws
are pre-scaled by 128 host-side to clear fp8 subnormals; the tanh/exp
activations apply scale=1/128 to compensate. X is shipped twice: fp8
(PE) and bf16 (VectorE context path). Total rel err ~1.7e-2 (sim-
verified; hardware matches the numpy fp8 sim to ~1e-5).

Per-core device layout (prepared host-side):
    xt8  [4, 4, 128, 4096] fp8  xt8[b,c,p,t*1024+r*512+s'] = X[b, c*512+s', (2t+r)*128+p]
    xtb  [4, 4, 128, 4096] bf16 xtb[b,c,p,k*512+s']        = X[b, c*512+s', k*128+p]
    we8  [8, 128, 1024]    fp8  we8[j,p,t*256+r*128+m]     = 128*We[(2t+r)*128+p, j*128+m]
    ws8  [128, 128]        fp8  ws8[p,(t*2+r)*16]          = 128*Ws[(2t+r)*128+p, 0]
                                (pair slabs padded to the 16B ldweights step)
    bias [128, 32]         f32  bias[p, j*4+b]             = (be+bd+dec[b]@Wd)[j*128+p]
    xs3  [128, 8192]       bf16 xs3[p, c'*4096+si*1024+n] = X[3, (2+c')*512+si*128+p, n]
                                (last TWO chunks of the last batch, s-major)
Outputs:
    ctx  [4, 128, 8] f32: batches 0-2 normalized contexts; batch 3 the
         UNNORMALIZED partial over chunks 0-1
    ctx3 [1, 1025]  f32: [:1024] batch 3's unnormalized chunk 2+3 partial
         (h on free); [1024] the softmax denominator
    (host: out[3] = (ctx[3].T + ctx3[:1024]) / ctx3[1024])

Device schedule per (batch, 512-wide s-chunk):
  - 8 j-groups x 4 DoubleRow matmuls accumulate enc_score.T in PSUM;
    ScalarE evacuates with fused tanh(psum/128 + bias) -> fp8 score
    pairs (j even/odd interleaved slabs for the ls DoubleRow rhs)
  - ls.T = 4 DoubleRow matmuls over score pairs -> PSUM [1,512]
  - the exp + context work for chunk c is DEFERRED into chunk c+1's
    matmul phase: the in-order ScalarE queue would otherwise stall on
    exp (which waits for the ls chain) ahead of the next chunk's tanh
    evacuations that the next ls matmuls gate on
  - context: exp weights broadcast to 128 partitions (GpSimd), one
    fused VectorE multiply [128,8,512] (broadcast-AP) + one fused
    per-k reduce -> ctx partials
  - kernel tail (last batch, last TWO chunks): the context runs on the
    PE instead of the DVE (whose fused reduce would spill past the
    matmul stream): exp -> PE transposes to [128,4] -> matmuls against
    the s-major bf16 chunks accumulating [1,1024] in shared PSUM,
    merged with the denominator on the host
  - startup: ws/bias then weight slabs split across both DMA rings,
    first chunk in contiguous k-pair slabs, and 12 full-array dummy
    matmuls during the DMA gate to pre-warm the PE clock gate
"""

import numpy as np
import ml_dtypes

import concourse.tile as tile
from concourse import bacc, mybir
from concourse.bass_utils import run_bass_kernel_spmd

BF16 = mybir.dt.bfloat16
F32 = mybir.dt.float32
FP8 = mybir.dt.float8e4
AF = mybir.ActivationFunctionType
DR = mybir.MatmulPerfMode.DoubleRow

N_CORES = 8
H = 1024
S = 2048
B_PER_CORE = 4
S_CHUNK = 512
WE_SCALE = 128.0

# test.py can flip this to get a profiled run; the grading path never does.
PROFILE = {"trace": False, "tmpdir": None}


def build_program(b_per_core=B_PER_CORE, s=S, h=H):
    kt = h // 128          # 8  bf16 k-tiles (context path)
    kt2 = kt // 2          # 4  fp8 DoubleRow k-pair tiles
    jt = h // 128          # 8  output h tiles
    jt2 = jt // 2          # 4  score pair tiles (ls DoubleRow)
    n_sc = s // S_CHUNK    # 4  s chunks
    nc = bacc.Bacc("TRN2", target_bir_lowering=False, debug=False)

    xt8_d = nc.dram_tensor(
        "xt8", [b_per_core, n_sc, 128, kt2 * 2 * S_CHUNK], FP8, kind="ExternalInput"
    ).ap()
    # first chunk duplicated t-major: each k-pair slab is contiguous so the
    # first matmul group can start per-slab
    xt8f_d = nc.dram_tensor("xt8f", [kt2, 128, 2 * S_CHUNK], FP8, kind="ExternalInput").ap()
    xtb_d = nc.dram_tensor(
        "xtb", [b_per_core, n_sc, 128, kt * S_CHUNK], BF16, kind="ExternalInput"
    ).ap()
    # j-major so each per-j weight slab is one contiguous 128KB transfer
    we8_d = nc.dram_tensor("we8", [jt, 128, h], FP8, kind="ExternalInput").ap()
    # ws pair slabs padded to 16 bytes: DoubleRow ldweights requires the
    # k-pair step to be a multiple of 16 bytes
    ws8_d = nc.dram_tensor("ws8", [128, jt2 * 2 * 16], FP8, kind="ExternalInput").ap()
    bias_d = nc.dram_tensor(
        "bias", [128, jt * b_per_core], F32, kind="ExternalInput"
    ).ap()
    xs3_d = nc.dram_tensor("xs3", [128, 2 * (S_CHUNK // 128) * h], BF16, kind="ExternalInput").ap()
    ctx_d = nc.dram_tensor("ctx", [b_per_core, 128, jt], F32, kind="ExternalOutput").ap()
    # ctx3[0, :h] = unnormalized tail-chunk context; ctx3[0, h] = denominator
    ctx3_d = nc.dram_tensor("ctx3", [1, h + 1], F32, kind="ExternalOutput").ap()

    with tile.TileContext(nc) as tc:
        with (
            tc.tile_pool(name="consts", bufs=1) as consts,
            tc.tile_pool(name="xp", bufs=8) as xp,
            tc.tile_pool(name="scorep", bufs=8) as scorep,
            tc.tile_pool(name="smallp", bufs=2 * n_sc) as smallp,
            tc.tile_pool(name="vp", bufs=3) as vp,
            tc.tile_pool(name="ctxp", bufs=4) as ctxp,
            tc.tile_pool(name="ps_main", bufs=4, space="PSUM") as ps_main,
            tc.tile_pool(name="ps_ls", bufs=3, space="PSUM") as ps_ls,
            tc.tile_pool(name="ps_misc", bufs=1, space="PSUM") as ps_misc,
        ):
            # Gate-opening DMAs run on BOTH rings in parallel: the first
            # chunk (one contiguous 512KB) on sync, the weight slabs
            # (contiguous 128KB each, j-major) on scalar. The first matmul
            # gates on xt8[0,0] + we[j=0] only.
            xt8_first = xp.tile([128, kt2, 2, S_CHUNK], FP8, tag="xt8")
            for t in range(kt2):
                nc.sync.dma_start(xt8_first[:, t], xt8f_d[t])
            # ws/bias go FIRST on the scalar ring: the tanh chain gates on
            # bias, and a bias queued behind 1MB of weight slabs once stalled
            # the whole second chunk for 11us
            ws_sb = consts.tile([128, jt2, 2, 16], FP8)
            nc.scalar.dma_start(ws_sb[:], ws8_d[:])
            bias_sb = consts.tile([128, jt * b_per_core], F32)
            nc.scalar.dma_start(bias_sb[:], bias_d[:])
            we_sb = consts.tile([128, jt, kt2, 2, 128], FP8)
            for j in range(0, jt, 2):
                nc.scalar.dma_start(we_sb[:, j], we8_d[j])
            for j in range(1, jt, 2):
                nc.sync.dma_start(we_sb[:, j], we8_d[j])
            xs_sb = consts.tile([128, 2, S_CHUNK // 128, h], BF16)
            ones_bf = consts.tile([1, 128], BF16)
            nc.vector.memset(ones_bf[:], 1.0)
            ones_f32 = consts.tile([1, 128], F32)
            nc.vector.memset(ones_f32[:], 1.0)
            # PE warmup during the startup DMA gate: full-array dummy
            # matmuls keep the HAM activity monitor busy so the real stream
            # starts at the full 2.4GHz instead of paying ~20 cold matmuls
            # at 1.2GHz (K=1 dummies don't move the activity counter)
            dum_w = consts.tile([128, 128], BF16)
            nc.vector.memset(dum_w[:], 1.0)
            dum_x = consts.tile([128, S_CHUNK], BF16)
            nc.vector.memset(dum_x[:], 1.0)
            warm_ps = ps_misc.tile([128, S_CHUNK], F32, tag="misc")
            for _ in range(14):
                nc.tensor.matmul(
                    warm_ps[:], lhsT=dum_w[:], rhs=dum_x[:], start=True, stop=True
                )

            def emit_exp(ls_ps, denom_b, c):
                """exp(ls/128) -> bf16 weights + f32 denominator slot."""
                ex = smallp.tile([1, S_CHUNK], BF16, tag="exp")
                nc.scalar.activation(
                    ex[:], ls_ps[:], AF.Exp, scale=1.0 / WE_SCALE,
                    accum_out=denom_b[:, c : c + 1],
                )
                return ex

            def emit_context_chunk(xtb_bc, ex, ctx4_b, c):
                """Broadcast chunk weights (GpSimd), then one fused multiply
                + one fused per-k reduce for the whole chunk (DVE)."""
                ebc = vp.tile([128, S_CHUNK], BF16, tag="ebc")
                nc.gpsimd.partition_broadcast(ebc[:], ex[:])
                scr = vp.tile([128, kt, S_CHUNK], BF16, tag="scr")
                ebc_b = ebc[:].unsqueeze(1).broadcast_to((128, kt, S_CHUNK))
                nc.vector.tensor_mul(scr[:], xtb_bc[:], ebc_b)
                # bf16 partials keep the reduce in the DVE's 2x perf mode
                # (2-byte dst required); costs ~0.2% relative error
                with nc.allow_low_precision("bf16 context partials"):
                    nc.vector.reduce_sum(
                        ctx4_b[:, c], scr[:], axis=mybir.AxisListType.X
                    )

            def emit_invd(denom_b, width):
                """softmax denominator -> broadcast 1/d [128, 1]."""
                dsum = smallp.tile([1, 1], F32, tag="dsum")
                nc.vector.reduce_sum(
                    dsum[:], denom_b[:, :width], axis=mybir.AxisListType.X
                )
                invd = smallp.tile([1, 1], F32, tag="invd")
                nc.vector.reciprocal(invd[:], dsum[:])
                iv_ps = ps_misc.tile([128, S_CHUNK], F32, tag="misc")
                nc.tensor.matmul(
                    iv_ps[:, 0:1], lhsT=ones_f32[:], rhs=invd[:], start=True, stop=True
                )
                invd_bc = smallp.tile([128, 1], F32, tag="invdbc")
                nc.scalar.copy(invd_bc[:], iv_ps[:, 0:1])
                return invd_bc

            def emit_batch_final(b, ctx4_b, invd_bc, width):
                """Partial reduction over chunks, normalize, store."""
                ctxu = ctxp.tile([128, jt], F32, tag="ctxu")
                nc.vector.reduce_sum(
                    ctxu[:],
                    ctx4_b[:, :width].transpose([0, 2, 1]),
                    axis=mybir.AxisListType.X,
                )
                if invd_bc is None:
                    nc.sync.dma_start(ctx_d[b], ctxu[:])
                else:
                    ctx_b = ctxp.tile([128, jt], F32, tag="ctx")
                    nc.vector.tensor_scalar_mul(ctx_b[:], ctxu[:], invd_bc[:])
                    nc.sync.dma_start(ctx_d[b], ctx_b[:])

            def emit_pe_ctx(ex_slices, xs_half, tailst, start):
                """One chunk of tail context on the PE: 4 transposes of the
                exp weights into exT [128,4], then 8 matmuls against the
                s-major bf16 chunk, accumulating [1,1024] across chunks in
                shared PSUM (start on the first chunk, stop on the last)."""
                exT_ps = ps_misc.tile([128, 4, 2], BF16, tag="misc")
                for si in range(4):
                    nc.tensor.transpose(
                        exT_ps[:, si, 0:1], ex_slices[si], ones_bf[:, 0:1]
                    )
                exT = smallp.tile([128, 4], BF16, tag="exT_sb")
                nc.scalar.copy(exT[:], exT_ps[:, :, 0])
                if start:
                    tailst["ctx_ps"] = [
                        ps_ls.tile([1, S_CHUNK], F32, tag="ls", name=f"ctx3ps{hh}")
                        for hh in range(2)
                    ]
                for hh in range(2):
                    c3_ps = tailst["ctx_ps"][hh]
                    for si in range(4):
                        nc.tensor.matmul(
                            c3_ps[:],
                            lhsT=exT[:, si : si + 1],
                            rhs=xs_half[:, si, hh * S_CHUNK : (hh + 1) * S_CHUNK],
                            start=(start and si == 0),
                            stop=((not start) and si == 3),
                            skip_group_check=True,
                        )

            def emit_pe_tail(ls_ps, denom_b, tailst):
                """Kernel tail: the last chunk's exp in two halves, its
                context on the idle PE, then the combined chunk-2+3 partial
                and the denominator ship for the host-side divide."""
                ex_h = []
                for hh in range(2):
                    exh = smallp.tile([1, 256], BF16, tag="exp")
                    nc.scalar.activation(
                        exh[:], ls_ps[:, hh * 256 : (hh + 1) * 256], AF.Exp,
                        scale=1.0 / WE_SCALE,
                        accum_out=denom_b[:, n_sc - 1 + hh : n_sc + hh],
                    )
                    ex_h.append(exh)
                slices = [
                    ex_h[si // 2][:, (si % 2) * 128 : (si % 2 + 1) * 128]
                    for si in range(4)
                ]
                emit_pe_ctx(slices, xs_sb[:, 1], tailst, start=False)
                ctx3_sb = ctxp.tile([1, h + 1], F32, tag="ctx3")
                nc.scalar.copy(ctx3_sb[:, :S_CHUNK], tailst["ctx_ps"][0][:])
                nc.vector.tensor_copy(
                    ctx3_sb[:, S_CHUNK : 2 * S_CHUNK], tailst["ctx_ps"][1][:]
                )
                # denominator: chunks 0-2 in slots 0-2, chunk 3 halves in 3-4
                nc.vector.reduce_sum(
                    ctx3_sb[:, h : h + 1], denom_b[:], axis=mybir.AxisListType.X
                )
                nc.sync.dma_start(ctx3_d[:], ctx3_sb[:])

            pending = []  # deferred (exp | context-chunk | invd | batch-final)
            pending_late = []  # deferred PE tail-context work (flushed at j==5)
            tailst = {}
            for b in range(b_per_core):
                last_b = b == b_per_core - 1
                if last_b:
                    nc.sync.dma_start(xs_sb[:], xs3_d[:])
                xt8_tiles = []
                xtb_tiles = []
                for c in range(n_sc):
                    if b == 0 and c == 0:
                        xt8_bc = xt8_first
                    else:
                        xt8_bc = xp.tile([128, kt2, 2, S_CHUNK], FP8, tag="xt8")
                        nc.sync.dma_start(xt8_bc[:], xt8_d[b, c])
                    xt8_tiles.append(xt8_bc)
                    # xtb rides the sync ring too: DMA issues on the scalar
                    # queue would steal ~1.8us/chunk from the tanh ACT chain
                    # that the ls matmuls gate on.
                    xtb_bc = xp.tile([128, kt, S_CHUNK], BF16, tag="xtb")
                    nc.sync.dma_start(xtb_bc[:], xtb_d[b, c])
                    xtb_tiles.append(xtb_bc)

                denom_b = smallp.tile([1, n_sc + 1], F32, tag="denom")
                ctx4_b = ctxp.tile([128, n_sc, kt], BF16, tag="ctx4")
                for c in range(n_sc):
                    ls_ps = ps_ls.tile([1, S_CHUNK], F32, tag="ls")
                    score_tiles = []
                    for j in range(jt):
                        mm_ps = ps_main.tile([128, S_CHUNK], F32, tag="main")
                        for t in range(kt2):
                            nc.tensor.matmul(
                                mm_ps[:],
                                lhsT=we_sb[:, j, t],
                                rhs=xt8_tiles[c][:, t],
                                start=(t == 0),
                                stop=(t == kt2 - 1),
                                perf_mode=DR,
                            )
                        if j % 2 == 0:
                            scp = scorep.tile([128, 2, S_CHUNK], FP8, tag="score")
                            score_tiles.append(scp)
                        nc.scalar.activation(
                            score_tiles[j // 2][:, j % 2], mm_ps[:], AF.Tanh,
                            bias=bias_sb[:, j * b_per_core + b : j * b_per_core + b + 1],
                            scale=1.0 / WE_SCALE,
                        )
                        if j == 2:
                            # deferred work from the previous chunk/batch is
                            # emitted two matmul groups in, so its ScalarE exp
                            # queues behind the tanh pair that the first ls
                            # matmul gates on, and the DVE context work
                            # overlaps this chunk's remaining matmul groups
                            for fn in pending:
                                fn()
                            pending = []
                        if j == 5 and pending_late:
                            # tail-context PE work flushes later still so its
                            # exp dependency has cleared the ScalarE queue
                            for fn in pending_late:
                                fn()
                            pending_late = []
                    for tj in range(jt2):
                        nc.tensor.matmul(
                            ls_ps[:],
                            lhsT=ws_sb[:, tj, :, 0:1],
                            rhs=score_tiles[tj][:],
                            start=(tj == 0),
                            stop=(tj == jt2 - 1),
                            perf_mode=DR,
                        )

                    if last_b and c == n_sc - 1:
                        # kernel tail: denominator + context via the idle PE
                        emit_pe_tail(ls_ps, denom_b, tailst)
                        emit_batch_final(b, ctx4_b, None, n_sc - 2)
                    elif last_b and c == n_sc - 2:
                        # the second-to-last chunk's context also runs on the
                        # PE (accumulating into the tail PSUM): its DVE reduce
                        # would otherwise spill ~5us past the matmul stream
                        def c2_exp(ls_ps=ls_ps, denom_b=denom_b, c=c):
                            tailst["ex2"] = emit_exp(ls_ps, denom_b, c)
                        pending.append(c2_exp)
                        def c2_pe():
                            ex2 = tailst["ex2"]
                            slices = [
                                ex2[:, si * 128 : (si + 1) * 128] for si in range(4)
                            ]
                            emit_pe_ctx(slices, xs_sb[:, 0], tailst, start=True)
                        pending_late.append(c2_pe)
                    elif c == n_sc - 1:
                        def batch_tail(ls_ps=ls_ps, b=b, c=c, ctx4_b=ctx4_b,
                                       denom_b=denom_b, xtb_bc=xtb_tiles[c]):
                            ex = emit_exp(ls_ps, denom_b, c)
                            invd_bc = emit_invd(denom_b, n_sc)
                            emit_context_chunk(xtb_bc, ex, ctx4_b, c)
                            emit_batch_final(b, ctx4_b, invd_bc, n_sc)
                        pending.append(batch_tail)
                    else:
                        def ctx_fn(ls_ps=ls_ps, c=c, ctx4_b=ctx4_b,
                                   denom_b=denom_b, xtb_bc=xtb_tiles[c]):
                            ex = emit_exp(ls_ps, denom_b, c)
                            emit_context_chunk(xtb_bc, ex, ctx4_b, c)
                        pending.append(ctx_fn)

    nc.compile()
    return nc


_CACHED = {}


def _get_program(key):
    if key not in _CACHED:
        _CACHED[key] = build_program(*key)
    return _CACHED[key]


def make_in_maps(encoder_out, decoder_hidden_state, We, be, Wd, bd, Ws, bs,
                 b_per_core=B_PER_CORE, s=S, h=H, n_cores=N_CORES):
    kt = h // 128
    kt2 = kt // 2
    jt = h // 128
    n_sc = s // S_CHUNK
    bf = ml_dtypes.bfloat16
    f8 = mybir.dt.np(FP8)

    # we8[j, p, t*256+r*128+m] = 128*We[(2t+r)*128+p, j*128+m]
    we8_a = np.ascontiguousarray(
        (We * WE_SCALE).reshape(kt2, 2, 128, jt, 128).transpose(3, 2, 0, 1, 4)
    ).reshape(jt, 128, h).astype(f8)
    # ws8[p, (t*2+r)*16] = 128*Ws[(2t+r)*128+p, 0]; 16-byte padded pair slabs
    ws8_a = np.zeros((128, kt2, 2, 16), dtype=f8)
    ws8_a[:, :, :, 0] = (
        (Ws[:, 0] * WE_SCALE).reshape(kt2, 2, 128).transpose(2, 0, 1).astype(f8)
    )
    ws8_a = ws8_a.reshape(128, kt2 * 2 * 16)

    dec = decoder_hidden_state[0]  # [32, h]
    bias_all = (be + bd)[None, :] + dec @ Wd  # [32, h] fp32
    in_maps = []
    for i in range(n_cores):
        b0 = i * b_per_core
        xb = encoder_out[b0 : b0 + b_per_core]  # [b, s, h]
        # fp8 PE copy: [b, c, s', t, r, p] -> [b, c, p, t, r, s']
        xt8_a = np.ascontiguousarray(
            xb.reshape(b_per_core, n_sc, S_CHUNK, kt2, 2, 128).transpose(0, 1, 5, 3, 4, 2)
        ).reshape(b_per_core, n_sc, 128, kt2 * 2 * S_CHUNK).astype(f8)
        # bf16 context copy: [b, c, s', k, p] -> [b, c, p, k, s']
        xtb_a = np.ascontiguousarray(
            xb.reshape(b_per_core, n_sc, S_CHUNK, kt, 128).transpose(0, 1, 4, 3, 2)
        ).reshape(b_per_core, n_sc, 128, kt * S_CHUNK).astype(bf)
        # s-major copy of the tail chunks (last batch, last TWO s-chunks),
        # p-major in DRAM so it ships as one contiguous DMA
        xs3_a = np.ascontiguousarray(
            xb[b_per_core - 1, (n_sc - 2) * S_CHUNK :]
            .reshape(2, S_CHUNK // 128, 128, h).transpose(2, 0, 1, 3)
        ).reshape(128, 2 * (S_CHUNK // 128) * h).astype(bf)
        # first chunk again, t-major contiguous slabs for the startup gate
        xt8f_a = np.ascontiguousarray(
            xt8_a[0, 0].reshape(128, kt2, 2 * S_CHUNK).transpose(1, 0, 2)
        )
        bias_a = np.ascontiguousarray(
            bias_all[b0 : b0 + b_per_core].reshape(b_per_core, jt, 128).transpose(2, 1, 0)
        ).reshape(128, jt * b_per_core).astype(np.float32)
        in_maps.append(
            {"xt8": xt8_a, "xtb": xtb_a, "we8": we8_a, "ws8": ws8_a,
             "bias": bias_a, "xs3": xs3_a, "xt8f": xt8f_a}
        )
    return in_maps


def kernel(encoder_out, decoder_hidden_state, We, be, Wd, bd, Ws, bs):
    encoder_out = np.asarray(encoder_out, dtype=np.float32)
    decoder_hidden_state = np.asarray(decoder_hidden_state, dtype=np.float32)
    We = np.asarray(We, dtype=np.float32)
    be = np.asarray(be, dtype=np.float32)
    Wd = np.asarray(Wd, dtype=np.float32)
    bd = np.asarray(bd, dtype=np.float32)
    Ws = np.asarray(Ws, dtype=np.float32)
    bs = np.asarray(bs, dtype=np.float32)

    nc = _get_program((B_PER_CORE, S, H))
    in_maps = make_in_maps(
        encoder_out, decoder_hidden_state, We, be, Wd, bd, Ws, bs
    )
    kwargs = {}
    if PROFILE["trace"]:
        kwargs = {"trace": True, "tmpdir": PROFILE["tmpdir"]}
    res = run_bass_kernel_spmd(nc, in_maps, list(range(N_CORES)), **kwargs)
    PROFILE["last_result"] = res

    out = np.empty((N_CORES * B_PER_CORE, H), dtype=np.float32)
    for i in range(N_CORES):
        r = res.results[i]
        ctx = r["ctx"]  # [b, 128, jt]
        out[i * B_PER_CORE : (i + 1) * B_PER_CORE] = (
            ctx.transpose(0, 2, 1).reshape(B_PER_CORE, H)
        )
        # batch 3: ctx[3] holds the unnormalized chunk 0-2 partial; add the
        # PE-tail chunk-3 partial and divide by the shipped denominator
        out[i * B_PER_CORE + B_PER_CORE - 1] = (
            ctx[B_PER_CORE - 1].T.reshape(H) + r["ctx3"][0, :H]
        ) / r["ctx3"][0, H]
    return out


# revision 22
# speedup vs baseline: 1.1751x; 1.1751x over previous
"""Bahdanau attention fused kernel for Trainium2, 8-core data-parallel.

Reference computation (per batch b of 32, H=1024, S=2048):
    enc_score = encoder_out @ We + be                    [B, S, H]
    dec_score = dec @ Wd + bd                            [B, 1, H]
    score     = tanh(enc_score + dec_score)              [B, S, H]
    ls        = score @ Ws + bs                          [B, S, 1]
    w         = softmax(ls, axis=S)
    out       = sum_s w[b,s] * encoder_out[b,s,:]        [B, H]

Sharding: batch 32 -> 4 per core across 8 cores; weights replicated.
The tiny dec-score GEMM is folded into the host-side bias preparation:
bias[b] = be + bd + dec[b] @ Wd. bs is dropped (softmax shift-invariant).

Numerics: the main GEMM and the ls projection run in fp8-e4m3 with
MatmulPerfMode.DoubleRow (two 128-deep k-tiles per matmul). We and Ws
are pre-scaled by 128 host-side to clear fp8 subnormals; the tanh/exp
activations apply scale=1/128 to compensate. X is shipped twice: fp8
(PE) and bf16 (VectorE context path). Total rel err ~1.7e-2 (sim-
verified; hardware matches the numpy fp8 sim to ~1e-5).

Per-core device layout (prepared host-side):
    xt8  [4, 4, 128, 4096] fp8  xt8[b,c,p,t*1024+r*512+s'] = X[b, c*512+s', (2t+r)*128+p]
    xtb  [4, 4, 128, 4096] bf16 xtb[b,c,p,k*512+s']        = X[b, c*512+s', k*128+p]
    we8  [8, 128, 1024]    fp8  we8[j,p,t*256+r*128+m]     = 128*We[(2t+r)*128+p, j*128+m]
    ws8  [128, 128]        fp8  ws8[p,(t*2+r)*16]          = 128*Ws[(2t+r)*128+p, 0]
                                (pair slabs padded to the 16B ldweights step)
    bias [128, 32]         f32  bias[p, j*4+b]             = (be+bd+dec[b]@Wd)[j*128+p]
    xs3  [128, 8192]       bf16 xs3[p, c'*4096+si*1024+n] = X[3, (2+c')*512+si*128+p, n]
                                (last TWO chunks of the last batch, s-major)
Outputs:
    ctx  [4, 128, 8] f32: batches 0-2 normalized contexts; batch 3 the
         UNNORMALIZED partial over chunks 0-1
    ctx3 [1, 1025]  f32: [:1024] batch 3's unnormalized chunk 2+3 partial
         (h on free); [1024] the softmax denominator
    (host: out[3] = (ctx[3].T + ctx3[:1024]) / ctx3[1024])

Device schedule per (batch, 512-wide s-chunk):
  - 8 j-groups x 4 DoubleRow matmuls accumulate enc_score.T in PSUM;
    ScalarE evacuates with fused tanh(psum/128 + bias) -> fp8 score
    pairs (j even/odd interleaved slabs for the ls DoubleRow rhs)
  - ls.T = 4 DoubleRow matmuls over score pairs -> PSUM [1,512]
  - the exp + context work for chunk c is DEFERRED into chunk c+1's
    matmul phase: the in-order ScalarE queue would otherwise stall on
    exp (which waits for the ls chain) ahead of the next chunk's tanh
    evacuations that the next ls matmuls gate on
  - context: exp weights broadcast to 128 partitions (GpSimd), one
    fused VectorE multiply [128,8,512] (broadcast-AP) + one fused
    per-k reduce -> ctx partials
  - kernel tail (last batch, last TWO chunks): the context runs on the
    PE instead of the DVE (whose fused reduce would spill past the
    matmul stream): exp -> PE transposes to [128,4] -> matmuls against
    the s-major bf16 chunks accumulating [1,1024] in shared PSUM,
    merged with the denominator on the host
  - startup: ws/bias then weight slabs split across both DMA rings,
    first chunk in contiguous k-pair slabs, and 12 full-array dummy
    matmuls during the DMA gate to pre-warm the PE clock gate
"""

import numpy as np
import ml_dtypes

import concourse.tile as tile
from concourse import bacc, mybir
from concourse.bass_utils import run_bass_kernel_spmd

BF16 = mybir.dt.bfloat16
F32 = mybir.dt.float32
FP8 = mybir.dt.float8e4
AF = mybir.ActivationFunctionType
DR = mybir.MatmulPerfMode.DoubleRow

N_CORES = 8
H = 1024
S = 2048
B_PER_CORE = 4
S_CHUNK = 512
WE_SCALE = 128.0

# test.py can flip this to get a profiled run; the grading path never does.
PROFILE = {"trace": False, "tmpdir": None}


def build_program(b_per_core=B_PER_CORE, s=S, h=H):
    kt = h // 128          # 8  bf16 k-tiles (context path)
    kt2 = kt // 2          # 4  fp8 DoubleRow k-pair tiles
    jt = h // 128          # 8  output h tiles
    jt2 = jt // 2          # 4  score pair tiles (ls DoubleRow)
    n_sc = s // S_CHUNK    # 4  s chunks
    nc = bacc.Bacc("TRN2", target_bir_lowering=False, debug=False)

    xt8_d = nc.dram_tensor(
        "xt8", [b_per_core, n_sc, 128, kt2 * 2 * S_CHUNK], FP8, kind="ExternalInput"
    ).ap()
    # first chunk duplicated t-major: each k-pair slab is contiguous so the
    # first matmul group can start per-slab
    xt8f_d = nc.dram_tensor("xt8f", [kt2, 128, 2 * S_CHUNK], FP8, kind="ExternalInput").ap()
    xtb_d = nc.dram_tensor(
        "xtb", [b_per_core, n_sc, 128, kt * S_CHUNK], BF16, kind="ExternalInput"
    ).ap()
    # j-major so each per-j weight slab is one contiguous 128KB transfer
    we8_d = nc.dram_tensor("we8", [jt, 128, h], FP8, kind="ExternalInput").ap()
    # ws pair slabs padded to 16 bytes: DoubleRow ldweights requires the
    # k-pair step to be a multiple of 16 bytes
    ws8_d = nc.dram_tensor("ws8", [128, jt2 * 2 * 16], FP8, kind="ExternalInput").ap()
    bias_d = nc.dram_tensor(
        "bias", [128, jt * b_per_core], F32, kind="ExternalInput"
    ).ap()
    xs3_d = nc.dram_tensor("xs3", [128, 2 * (S_CHUNK // 128) * h], BF16, kind="ExternalInput").ap()
    ctx_d = nc.dram_tensor("ctx", [b_per_core, 128, jt], F32, kind="ExternalOutput").ap()
    # ctx3[0, :h] = unnormalized tail-chunk context; ctx3[0, h] = denominator
    ctx3_d = nc.dram_tensor("ctx3", [1, h + 1], F32, kind="ExternalOutput").ap()

    with tile.TileContext(nc) as tc:
        with (
            tc.tile_pool(name="consts", bufs=1) as consts,
            tc.tile_pool(name="xp", bufs=8) as xp,
            tc.tile_pool(name="scorep", bufs=8) as scorep,
            tc.tile_pool(name="smallp", bufs=2 * n_sc) as smallp,
            tc.tile_pool(name="vp", bufs=3) as vp,
            tc.tile_pool(name="ctxp", bufs=4) as ctxp,
            tc.tile_pool(name="ps_main", bufs=4, space="PSUM") as ps_main,
            tc.tile_pool(name="ps_ls", bufs=3, space="PSUM") as ps_ls,
            tc.tile_pool(name="ps_misc", bufs=1, space="PSUM") as ps_misc,
        ):
            # Gate-opening DMAs run on BOTH rings in parallel: the first
            # chunk (one contiguous 512KB) on sync, the weight slabs
            # (contiguous 128KB each, j-major) on scalar. The first matmul
            # gates on xt8[0,0] + we[j=0] only.
            xt8_first = xp.tile([128, kt2, 2, S_CHUNK], FP8, tag="xt8")
            for t in range(kt2):
                nc.sync.dma_start(xt8_first[:, t], xt8f_d[t])
            # ws/bias go FIRST on the scalar ring: the tanh chain gates on
            # bias, and a bias queued behind 1MB of weight slabs once stalled
            # the whole second chunk for 11us
            ws_sb = consts.tile([128, jt2, 2, 16], FP8)
            nc.scalar.dma_start(ws_sb[:], ws8_d[:])
            bias_sb = consts.tile([128, jt * b_per_core], F32)
            nc.scalar.dma_start(bias_sb[:], bias_d[:])
            we_sb = consts.tile([128, jt, kt2, 2, 128], FP8)
            for j in range(0, jt, 2):
                nc.scalar.dma_start(we_sb[:, j], we8_d[j])
            for j in range(1, jt, 2):
                nc.sync.dma_start(we_sb[:, j], we8_d[j])
            xs_sb = consts.tile([128, 2, S_CHUNK // 128, h], BF16)
            ones_bf = consts.tile([1, 128], BF16)
            nc.vector.memset(ones_bf[:], 1.0)
            ones_f32 = consts.tile([1, 128], F32)
            nc.vector.memset(ones_f32[:], 1.0)
            # PE warmup during the startup DMA gate: full-array dummy
            # matmuls keep the HAM activity monitor busy so the real stream
            # starts at the full 2.4GHz instead of paying ~20 cold matmuls
            # at 1.2GHz (K=1 dummies don't move the activity counter)
            dum_w = consts.tile([128, 128], BF16)
            nc.vector.memset(dum_w[:], 0.0)
            dum_x = consts.tile([128, S_CHUNK], BF16)
            nc.vector.memset(dum_x[:], 0.0)
            warm_ps = ps_misc.tile([128, S_CHUNK], F32, tag="misc")
            for _ in range(12):
                nc.tensor.matmul(
                    warm_ps[:], lhsT=dum_w[:], rhs=dum_x[:], start=True, stop=True
                )

            def emit_exp(ls_ps, denom_b, c):
                """exp(ls/128) -> bf16 weights + f32 denominator slot."""
                ex = smallp.tile([1, S_CHUNK], BF16, tag="exp")
                nc.scalar.activation(
                    ex[:], ls_ps[:], AF.Exp, scale=1.0 / WE_SCALE,
                    accum_out=denom_b[:, c : c + 1],
                )
                return ex

            def emit_context_chunk(xtb_bc, ex, ctx4_b, c):
                """Broadcast chunk weights (GpSimd), then one fused multiply
                + one fused per-k reduce for the whole chunk (DVE)."""
                ebc = vp.tile([128, S_CHUNK], BF16, tag="ebc")
                nc.gpsimd.partition_broadcast(ebc[:], ex[:])
                scr = vp.tile([128, kt, S_CHUNK], BF16, tag="scr")
                ebc_b = ebc[:].unsqueeze(1).broadcast_to((128, kt, S_CHUNK))
                nc.vector.tensor_mul(scr[:], xtb_bc[:], ebc_b)
                # bf16 partials keep the reduce in the DVE's 2x perf mode
                # (2-byte dst required); costs ~0.2% relative error
                with nc.allow_low_precision("bf16 context partials"):
                    nc.vector.reduce_sum(
                        ctx4_b[:, c], scr[:], axis=mybir.AxisListType.X
                    )

            def emit_invd(denom_b, width):
                """softmax denominator -> broadcast 1/d [128, 1]."""
                dsum = smallp.tile([1, 1], F32, tag="dsum")
                nc.vector.reduce_sum(
                    dsum[:], denom_b[:, :width], axis=mybir.AxisListType.X
                )
                invd = smallp.tile([1, 1], F32, tag="invd")
                nc.vector.reciprocal(invd[:], dsum[:])
                iv_ps = ps_misc.tile([128, S_CHUNK], F32, tag="misc")
                nc.tensor.matmul(
                    iv_ps[:, 0:1], lhsT=ones_f32[:], rhs=invd[:], start=True, stop=True
                )
                invd_bc = smallp.tile([128, 1], F32, tag="invdbc")
                nc.scalar.copy(invd_bc[:], iv_ps[:, 0:1])
                return invd_bc

            def emit_batch_final(b, ctx4_b, invd_bc, width):
                """Partial reduction over chunks, normalize, store."""
                ctxu = ctxp.tile([128, jt], F32, tag="ctxu")
                nc.vector.reduce_sum(
                    ctxu[:],
                    ctx4_b[:, :width].transpose([0, 2, 1]),
                    axis=mybir.AxisListType.X,
                )
                if invd_bc is None:
                    nc.sync.dma_start(ctx_d[b], ctxu[:])
                else:
                    ctx_b = ctxp.tile([128, jt], F32, tag="ctx")
                    nc.vector.tensor_scalar_mul(ctx_b[:], ctxu[:], invd_bc[:])
                    nc.sync.dma_start(ctx_d[b], ctx_b[:])

            def emit_pe_ctx(ex_slices, xs_half, tailst, start):
                """One chunk of tail context on the PE: 4 transposes of the
                exp weights into exT [128,4], then 8 matmuls against the
                s-major bf16 chunk, accumulating [1,1024] across chunks in
                shared PSUM (start on the first chunk, stop on the last)."""
                exT_ps = ps_misc.tile([128, 4, 2], BF16, tag="misc")
                for si in range(4):
                    nc.tensor.transpose(
                        exT_ps[:, si, 0:1], ex_slices[si], ones_bf[:, 0:1]
                    )
                exT = smallp.tile([128, 4], BF16, tag="exT_sb")
                nc.scalar.copy(exT[:], exT_ps[:, :, 0])
                if start:
                    tailst["ctx_ps"] = [
                        ps_ls.tile([1, S_CHUNK], F32, tag="ls", name=f"ctx3ps{hh}")
                        for hh in range(2)
                    ]
                for hh in range(2):
                    c3_ps = tailst["ctx_ps"][hh]
                    for si in range(4):
                        nc.tensor.matmul(
                            c3_ps[:],
                            lhsT=exT[:, si : si + 1],
                            rhs=xs_half[:, si, hh * S_CHUNK : (hh + 1) * S_CHUNK],
                            start=(start and si == 0),
                            stop=((not start) and si == 3),
                            skip_group_check=True,
                        )

            def emit_pe_tail(ls_ps, denom_b, tailst):
                """Kernel tail: the last chunk's exp in two halves, its
                context on the idle PE, then the combined chunk-2+3 partial
                and the denominator ship for the host-side divide."""
                ex_h = []
                for hh in range(2):
                    exh = smallp.tile([1, 256], BF16, tag="exp")
                    nc.scalar.activation(
                        exh[:], ls_ps[:, hh * 256 : (hh + 1) * 256], AF.Exp,
                        scale=1.0 / WE_SCALE,
                        accum_out=denom_b[:, n_sc - 1 + hh : n_sc + hh],
                    )
                    ex_h.append(exh)
                slices = [
                    ex_h[si // 2][:, (si % 2) * 128 : (si % 2 + 1) * 128]
                    for si in range(4)
                ]
                emit_pe_ctx(slices, xs_sb[:, 1], tailst, start=False)
                ctx3_sb = ctxp.tile([1, h + 1], F32, tag="ctx3")
                nc.scalar.copy(ctx3_sb[:, :S_CHUNK], tailst["ctx_ps"][0][:])
                nc.vector.tensor_copy(
                    ctx3_sb[:, S_CHUNK : 2 * S_CHUNK], tailst["ctx_ps"][1][:]
                )
                # denominator: chunks 0-2 in slots 0-2, chunk 3 halves in 3-4
                nc.vector.reduce_sum(
                    ctx3_sb[:, h : h + 1], denom_b[:], axis=mybir.AxisListType.X
                )
                nc.sync.dma_start(ctx3_d[:], ctx3_sb[:])

            pending = []  # deferred (exp | context-chunk | invd | batch-final)
            pending_late = []  # deferred PE tail-context work (flushed at j==5)
            tailst = {}
            for b in range(b_per_core):
                last_b = b == b_per_core - 1
                if last_b:
                    nc.sync.dma_start(xs_sb[:], xs3_d[:])
                xt8_tiles = []
                xtb_tiles = []
                for c in range(n_sc):
                    if b == 0 and c == 0:
                        xt8_bc = xt8_first
                    else:
                        xt8_bc = xp.tile([128, kt2, 2, S_CHUNK], FP8, tag="xt8")
                        nc.sync.dma_start(xt8_bc[:], xt8_d[b, c])
                    xt8_tiles.append(xt8_bc)
                    # xtb rides the sync ring too: DMA issues on the scalar
                    # queue would steal ~1.8us/chunk from the tanh ACT chain
                    # that the ls matmuls gate on.
                    xtb_bc = xp.tile([128, kt, S_CHUNK], BF16, tag="xtb")
                    nc.sync.dma_start(xtb_bc[:], xtb_d[b, c])
                    xtb_tiles.append(xtb_bc)

                denom_b = smallp.tile([1, n_sc + 1], F32, tag="denom")
                ctx4_b = ctxp.tile([128, n_sc, kt], BF16, tag="ctx4")
                for c in range(n_sc):
                    ls_ps = ps_ls.tile([1, S_CHUNK], F32, tag="ls")
                    score_tiles = []
                    for j in range(jt):
                        mm_ps = ps_main.tile([128, S_CHUNK], F32, tag="main")
                        for t in range(kt2):
                            nc.tensor.matmul(
                                mm_ps[:],
                                lhsT=we_sb[:, j, t],
                                rhs=xt8_tiles[c][:, t],
                                start=(t == 0),
                                stop=(t == kt2 - 1),
                                perf_mode=DR,
                            )
                        if j % 2 == 0:
                            scp = scorep.tile([128, 2, S_CHUNK], FP8, tag="score")
                            score_tiles.append(scp)
                        nc.scalar.activation(
                            score_tiles[j // 2][:, j % 2], mm_ps[:], AF.Tanh,
                            bias=bias_sb[:, j * b_per_core + b : j * b_per_core + b + 1],
                            scale=1.0 / WE_SCALE,
                        )
                        if j == 2:
                            # deferred work from the previous chunk/batch is
                            # emitted two matmul groups in, so its ScalarE exp
                            # queues behind the tanh pair that the first ls
                            # matmul gates on, and the DVE context work
                            # overlaps this chunk's remaining matmul groups
                            for fn in pending:
                                fn()
                            pending = []
                        if j == 5 and pending_late:
                            # tail-context PE work flushes later still so its
                            # exp dependency has cleared the ScalarE queue
                            for fn in pending_late:
                                fn()
                            pending_late = []
                    for tj in range(jt2):
                        nc.tensor.matmul(
                            ls_ps[:],
                            lhsT=ws_sb[:, tj, :, 0:1],
                            rhs=score_tiles[tj][:],
                            start=(tj == 0),
                            stop=(tj == jt2 - 1),
                            perf_mode=DR,
                        )

                    if last_b and c == n_sc - 1:
                        # kernel tail: denominator + context via the idle PE
                        emit_pe_tail(ls_ps, denom_b, tailst)
                        emit_batch_final(b, ctx4_b, None, n_sc - 2)
                    elif last_b and c == n_sc - 2:
                        # the second-to-last chunk's context also runs on the
                        # PE (accumulating into the tail PSUM): its DVE reduce
                        # would otherwise spill ~5us past the matmul stream
                        def c2_exp(ls_ps=ls_ps, denom_b=denom_b, c=c):
                            tailst["ex2"] = emit_exp(ls_ps, denom_b, c)
                        pending.append(c2_exp)
                        def c2_pe():
                            ex2 = tailst["ex2"]
                            slices = [
                                ex2[:, si * 128 : (si + 1) * 128] for si in range(4)
                            ]
                            emit_pe_ctx(slices, xs_sb[:, 0], tailst, start=True)
                        pending_late.append(c2_pe)
                    elif c == n_sc - 1:
                        def batch_tail(ls_ps=ls_ps, b=b, c=c, ctx4_b=ctx4_b,
                                       denom_b=denom_b, xtb_bc=xtb_tiles[c]):
                            ex = emit_exp(ls_ps, denom_b, c)
                            invd_bc = emit_invd(denom_b, n_sc)
                            emit_context_chunk(xtb_bc, ex, ctx4_b, c)
                            emit_batch_final(b, ctx4_b, invd_bc, n_sc)
                        pending.append(batch_tail)
                    else:
                        def ctx_fn(ls_ps=ls_ps, c=c, ctx4_b=ctx4_b,
                                   denom_b=denom_b, xtb_bc=xtb_tiles[c]):
                            ex = emit_exp(ls_ps, denom_b, c)
                            emit_context_chunk(xtb_bc, ex, ctx4_b, c)
                        pending.append(ctx_fn)

    nc.compile()
    return nc


_CACHED = {}


def _get_program(key):
    if key not in _CACHED:
        _CACHED[key] = build_program(*key)
    return _CACHED[key]


def make_in_maps(encoder_out, decoder_hidden_state, We, be, Wd, bd, Ws, bs,
                 b_per_core=B_PER_CORE, s=S, h=H, n_cores=N_CORES):
    kt = h // 128
    kt2 = kt // 2
    jt = h // 128
    n_sc = s // S_CHUNK
    bf = ml_dtypes.bfloat16
    f8 = mybir.dt.np(FP8)

    # we8[j, p, t*256+r*128+m] = 128*We[(2t+r)*128+p, j*128+m]
    we8_a = np.ascontiguousarray(
        (We * WE_SCALE).reshape(kt2, 2, 128, jt, 128).transpose(3, 2, 0, 1, 4)
    ).reshape(jt, 128, h).astype(f8)
    # ws8[p, (t*2+r)*16] = 128*Ws[(2t+r)*128+p, 0]; 16-byte padded pair slabs
    ws8_a = np.zeros((128, kt2, 2, 16), dtype=f8)
    ws8_a[:, :, :, 0] = (
        (Ws[:, 0] * WE_SCALE).reshape(kt2, 2, 128).transpose(2, 0, 1).astype(f8)
    )
    ws8_a = ws8_a.reshape(128, kt2 * 2 * 16)

    dec = decoder_hidden_state[0]  # [32, h]
    bias_all = (be + bd)[None, :] + dec @ Wd  # [32, h] fp32
    in_maps = []
    for i in range(n_cores):
        b0 = i * b_per_core
        xb = encoder_out[b0 : b0 + b_per_core]  # [b, s, h]
        # fp8 PE copy: [b, c, s', t, r, p] -> [b, c, p, t, r, s']
        xt8_a = np.ascontiguousarray(
            xb.reshape(b_per_core, n_sc, S_CHUNK, kt2, 2, 128).transpose(0, 1, 5, 3, 4, 2)
        ).reshape(b_per_core, n_sc, 128, kt2 * 2 * S_CHUNK).astype(f8)
        # bf16 context copy: [b, c, s', k, p] -> [b, c, p, k, s']
        xtb_a = np.ascontiguousarray(
            xb.reshape(b_per_core, n_sc, S_CHUNK, kt, 128).transpose(0, 1, 4, 3, 2)
        ).reshape(b_per_core, n_sc, 128, kt * S_CHUNK).astype(bf)
        # s-major copy of the tail chunks (last batch, last TWO s-chunks),
        # p-major in DRAM so it ships as one contiguous DMA
        xs3_a = np.ascontiguousarray(
            xb[b_per_core - 1, (n_sc - 2) * S_CHUNK :]
            .reshape(2, S_CHUNK // 128, 128, h).transpose(2, 0, 1, 3)
        ).reshape(128, 2 * (S_CHUNK // 128) * h).astype(bf)
        # first chunk again, t-major contiguous slabs for the startup gate
        xt8f_a = np.ascontiguousarray(
            xt8_a[0, 0].reshape(128, kt2, 2 * S_CHUNK).transpose(1, 0, 2)
        )
        bias_a = np.ascontiguousarray(
            bias_all[b0 : b0 + b_per_core].reshape(b_per_core, jt, 128).transpose(2, 1, 0)
        ).reshape(128, jt * b_per_core).astype(np.float32)
        in_maps.append(
            {"xt8": xt8_a, "xtb": xtb_a, "we8": we8_a, "ws8": ws8_a,
             "bias": bias_a, "xs3": xs3_a, "xt8f": xt8f_a}
        )
    return in_maps


def kernel(encoder_out, decoder_hidden_state, We, be, Wd, bd, Ws, bs):
    encoder_out = np.asarray(encoder_out, dtype=np.float32)
    decoder_hidden_state = np.asarray(decoder_hidden_state, dtype=np.float32)
    We = np.asarray(We, dtype=np.float32)
    be = np.asarray(be, dtype=np.float32)
    Wd = np.asarray(Wd, dtype=np.float32)
    bd = np.asarray(bd, dtype=np.float32)
    Ws = np.asarray(Ws, dtype=np.float32)
    bs = np.asarray(bs, dtype=np.float32)

    nc = _get_program((B_PER_CORE, S, H))
    in_maps = make_in_maps(
        encoder_out, decoder_hidden_state, We, be, Wd, bd, Ws, bs
    )
    kwargs = {}
    if PROFILE["trace"]:
        kwargs = {"trace": True, "tmpdir": PROFILE["tmpdir"]}
    res = run_bass_kernel_spmd(nc, in_maps, list(range(N_CORES)), **kwargs)
    PROFILE["last_result"] = res

    out = np.empty((N_CORES * B_PER_CORE, H), dtype=np.float32)
    for i in range(N_CORES):
        r = res.results[i]
        ctx = r["ctx"]  # [b, 128, jt]
        out[i * B_PER_CORE : (i + 1) * B_PER_CORE] = (
            ctx.transpose(0, 2, 1).reshape(B_PER_CORE, H)
        )
        # batch 3: ctx[3] holds the unnormalized chunk 0-2 partial; add the
        # PE-tail chunk-3 partial and divide by the shipped denominator
        out[i * B_PER_CORE + B_PER_CORE - 1] = (
            ctx[B_PER_CORE - 1].T.reshape(H) + r["ctx3"][0, :H]
        ) / r["ctx3"][0, H]
    return out
